# revision 9
# baseline (speedup 1.0000x reference)
"""Trainium2 Bass kernel for nn_BERT4GCN_53884659695997.

Mathematical reduction
----------------------
In the reference, ``feature`` is reassigned to ``LN(guidance)`` at the top of
every loop iteration, so the GCN block's output is never consumed; only the
last BERT layer's branch (index 3 -> hidden_states layer 12, which skips the
GCN block) reaches the output:

    t[b]      = LN(relu(hs[12,b][ts[b]] @ guid_W[3] + guid_b[3])) * ln_g + ln_b
    logits[b] = ((t[b] * m[b,:,None]).sum(0) / m[b].sum()) @ cls_W + cls_b

(verified numerically against the jax reference to ~7e-7 rel err).

Row gathers commute with the row-wise ops (matmul-by-row / relu / LN), so we
compute on the 256 *source* rows and fold gather+mask into per-source-row
weights  w[r] = sum_i m[i] * [ts[i] == r],  built on device via a one-hot
matmul.  ln_g / ln_b are folded into cls_W / cls_b host-side (exact linear
algebra in fp32).

Sharding: data-parallel over batch B=64 -> 8 samples per core on 8 cores.
All device arithmetic is fp32; PE matmuls accumulate in fp32 PSUM.
"""

import numpy as np
from contextlib import ExitStack

import concourse.bass as bass
import concourse.tile as tile
from concourse import bacc, mybir
from concourse.bass_utils import run_bass_kernel_spmd

F32 = mybir.dt.float32
AX = mybir.AxisListType
ALU = mybir.AluOpType
ACTF = mybir.ActivationFunctionType

N_CORES = 8
B = 64          # full batch
BC = B // N_CORES   # samples per core
L = 256         # tokens
D = 768         # bert dim
H = 600         # hidden*2
EPS = 1e-5
KT = D // 128   # 6 k-tiles
IT = L // 128   # 2 row-tiles
NSPLIT = (0, 300, 600)          # psum-bank-safe N split of H
HCH = ((0, 128), (128, 256), (256, 384), (384, 512), (512, 600))  # h chunks


def build_program(repeats: int = 1, stage: int = 99):
    nc = bacc.Bacc("TRN2", target_bir_lowering=False, debug=False,
                   num_devices=N_CORES)

    dr = {}
    def din(name, shape):
        dr[name] = nc.dram_tensor(name, list(shape), F32, kind="ExternalInput").ap()
    din("hs", (BC, L, D))
    din("gw", (D, H))
    din("gbrep", (128, H))
    din("tsf", (L, BC))
    din("mT", (L, BC))
    din("mnat", (BC, L))
    din("iota", (128, L))
    din("eye", (128, 128))
    din("clsw", (640, 3))      # ln_g-folded cls_W, zero-padded 600->640
    din("clsb", (BC, 3))       # ln_b@cls_W + cls_b, replicated rows
    out_ap = nc.dram_tensor("out", [BC, 3], F32, kind="ExternalOutput").ap()

    with tile.TileContext(nc) as tc, ExitStack() as ctx:
        cpool = ctx.enter_context(tc.tile_pool(name="consts", bufs=1))
        hpool = ctx.enter_context(tc.tile_pool(name="hs", bufs=2))
        tpool = ctx.enter_context(tc.tile_pool(name="hst", bufs=2))
        apool = ctx.enter_context(tc.tile_pool(name="act", bufs=3))
        npool = ctx.enter_context(tc.tile_pool(name="norm", bufs=2))
        spool = ctx.enter_context(tc.tile_pool(name="small", bufs=2))
        stats = ctx.enter_context(tc.tile_pool(name="stats", bufs=1))
        pg_ps = ctx.enter_context(tc.tile_pool(name="pg", bufs=4, space="PSUM"))
        sm_ps = ctx.enter_context(tc.tile_pool(name="sm", bufs=3, space="PSUM"))
        asp_ps = ctx.enter_context(tc.tile_pool(name="asp", bufs=1, space="PSUM"))

        # ---- constants (loaded once) ----
        GW = cpool.tile([128, KT, H], F32, tag="gw")
        nc.sync.dma_start(GW[:], dr["gw"].rearrange("(k p) n -> p k n", p=128))
        GB = cpool.tile([128, H], F32, tag="gb")
        nc.sync.dma_start(GB[:], dr["gbrep"][:])
        IOTA = cpool.tile([128, L], F32, tag="iota")
        nc.sync.dma_start(IOTA[:], dr["iota"][:])
        EYE = cpool.tile([128, 128], F32, tag="eye")
        nc.sync.dma_start(EYE[:], dr["eye"][:])
        TSF = cpool.tile([128, IT, BC], F32, tag="tsf")
        nc.sync.dma_start(TSF[:], dr["tsf"].rearrange("(t p) s -> p t s", p=128))
        MT = cpool.tile([128, IT, BC], F32, tag="mt")
        nc.sync.dma_start(MT[:], dr["mT"].rearrange("(t p) s -> p t s", p=128))
        MN = cpool.tile([BC, L], F32, tag="mn")
        nc.sync.dma_start(MN[:], dr["mnat"][:])
        CLSW = cpool.tile([128, 5, 3], F32, tag="clsw")
        nc.sync.dma_start(CLSW[:], dr["clsw"].rearrange("(c p) n -> p c n", p=128))
        CLSB = cpool.tile([BC, 3], F32, tag="clsb")
        nc.sync.dma_start(CLSB[:], dr["clsb"][:])

        # 1/sum(m) per sample: [BC,1]
        SM = stats.tile([BC, 1], F32, tag="sm")
        nc.vector.tensor_reduce(SM[:], MN[:], AX.X, ALU.add)
        RECIP = stats.tile([BC, 1], F32, tag="recip")
        nc.vector.reciprocal(RECIP[:], SM[:])

        # LN stats accumulators, one column per (sample, row-tile)
        S1 = stats.tile([128, 2 * BC], F32, tag="s1")
        S2 = stats.tile([128, 2 * BC], F32, tag="s2")
        MU = stats.tile([128, 2 * BC], F32, tag="mu")
        RS = stats.tile([128, 2 * BC], F32, tag="rs")

        def probe_out(src_ap):
            OSB = stats.tile([BC, 3], F32, tag="osb")
            nc.vector.tensor_copy(OSB[:], src_ap)
            nc.vector.tensor_add(OSB[:], OSB[:], CLSB[:])
            nc.sync.dma_start(out_ap[:], OSB[:])

        def body():
            probe = None
            if stage == 0:
                probe_out(MN[:BC, 0:3])
                return
            ASPT = asp_ps.tile([128, 5 * BC], F32, tag="aspt")
            for s in range(BC):
                # -------- load sample, transpose to [d, i] layout --------
                HSN = hpool.tile([128, IT, D], F32, tag="hsn")
                nc.sync.dma_start(HSN[:], dr["hs"][s].rearrange("(t p) d -> p t d", p=128))
                if stage == 1:
                    probe = HSN[:BC, 0, 0:3]
                    continue
                HST = tpool.tile([128, KT, L], F32, tag="hst")
                for kt in range(KT):
                    for it in range(IT):
                        PT = sm_ps.tile([128, 128], F32, tag="sm")
                        nc.tensor.transpose(PT[:], HSN[:, it, kt * 128:(kt + 1) * 128], EYE[:])
                        nc.scalar.copy(HST[:, kt, it * 128:(it + 1) * 128], PT[:])
                if stage == 2:
                    probe = HST[:BC, 0, 0:3]
                    continue

                # -------- guidance matmul + bias + relu + stats --------
                TN = npool.tile([128, IT, H], F32, tag="tn")
                for mt in range(IT):
                    col = s * 2 + mt
                    PGs = []
                    for ni in range(len(NSPLIT) - 1):
                        nlo, nhi = NSPLIT[ni], NSPLIT[ni + 1]
                        PG = pg_ps.tile([128, nhi - nlo], F32, tag="pg")
                        for kt in range(KT):
                            nc.tensor.matmul(
                                PG[:], HST[:, kt, mt * 128:(mt + 1) * 128],
                                GW[:, kt, nlo:nhi],
                                start=(kt == 0), stop=(kt == KT - 1))
                        PGs.append(PG)
                    T0 = apool.tile([128, H], F32, tag="t0")
                    for ni in range(len(NSPLIT) - 1):
                        nlo, nhi = NSPLIT[ni], NSPLIT[ni + 1]
                        nc.vector.tensor_add(T0[:, nlo:nhi], PGs[ni][:], GB[:, nlo:nhi])
                    if stage == 30:
                        probe = T0[:BC, 0:3]
                        continue
                    GR = apool.tile([128, H], F32, tag="gr")
                    if stage == 31:
                        nc.scalar.activation(GR[:], T0[:], ACTF.Relu)
                        probe = GR[:BC, 0:3]
                        continue
                    nc.scalar.activation(GR[:], T0[:], ACTF.Relu,
                                         accum_out=S1[:, col:col + 1])
                    if stage == 32:
                        probe = S1[:BC, 0:3]
                        continue
                    SQ = apool.tile([128, H], F32, tag="sq")
                    nc.scalar.activation(SQ[:], GR[:], ACTF.Square,
                                         accum_out=S2[:, col:col + 1])
                    if stage == 33:
                        probe = S2[:BC, 0:3]
                        continue
                    # stats -> mu, rstd for this column
                    c2 = (col, col + 1)
                    nc.vector.tensor_scalar_mul(MU[:, c2[0]:c2[1]], S1[:, c2[0]:c2[1]], 1.0 / H)
                    V = spool.tile([128, 1], F32, tag="v")
                    nc.vector.tensor_scalar_mul(V[:], S2[:, c2[0]:c2[1]], 1.0 / H)
                    MSQ = spool.tile([128, 1], F32, tag="msq")
                    nc.vector.tensor_mul(MSQ[:], MU[:, c2[0]:c2[1]], MU[:, c2[0]:c2[1]])
                    nc.vector.tensor_sub(V[:], V[:], MSQ[:])
                    nc.vector.tensor_scalar_add(V[:], V[:], EPS)
                    SD = spool.tile([128, 1], F32, tag="sd")
                    nc.scalar.sqrt(SD[:], V[:])
                    nc.vector.reciprocal(RS[:, c2[0]:c2[1]], SD[:])
                    # normalize
                    nc.vector.tensor_scalar(
                        TN[:, mt, :], GR[:], MU[:, col:col + 1], RS[:, col:col + 1],
                        ALU.subtract, ALU.mult)

                if stage in (30, 31, 32, 33):
                    continue
                if stage == 3:
                    probe = TN[:BC, 0, 0:3]
                    continue
                # -------- gather weights w[r] = sum_i m[i][ts[i]==r] --------
                WPS = sm_ps.tile([128, IT], F32, tag="sm")
                SOHs = []
                for it in range(IT):
                    SOH = spool.tile([128, L], F32, tag="soh")
                    nc.vector.tensor_scalar(SOH[:], IOTA[:], TSF[:, it, s:s + 1], None,
                                            ALU.is_equal)
                    SOHs.append(SOH)
                for rt in range(IT):
                    for it in range(IT):
                        nc.tensor.matmul(
                            WPS[:, rt:rt + 1], SOHs[it][:, rt * 128:(rt + 1) * 128],
                            MT[:, it, s:s + 1],
                            start=(it == 0), stop=(it == IT - 1))
                WSB = spool.tile([128, IT], F32, tag="wsb")
                nc.vector.tensor_copy(WSB[:], WPS[:])

                if stage == 4:
                    probe = WSB[:BC, 0:1]
                    continue
                # -------- aspects^T column s (unscaled masked sums) --------
                for hc, (hlo, hhi) in enumerate(HCH):
                    for it in range(IT):
                        nc.tensor.matmul(
                            ASPT[:hhi - hlo, hc * BC + s:hc * BC + s + 1],
                            TN[:, it, hlo:hhi], WSB[:, it:it + 1],
                            start=(it == 0), stop=(it == IT - 1))

            if probe is not None:
                if probe.shape[-1] != 3:
                    OSB = stats.tile([BC, 3], F32, tag="osb")
                    nc.vector.tensor_copy(OSB[:], CLSB[:])
                    nc.vector.tensor_copy(OSB[:, 0:1], probe)
                    nc.sync.dma_start(out_ap[:], OSB[:])
                else:
                    probe_out(probe)
                return
            # -------- classifier --------
            ASB = stats.tile([128, 5 * BC], F32, tag="asb")
            for hc, (hlo, hhi) in enumerate(HCH):
                sz = hhi - hlo
                nc.scalar.copy(ASB[:sz, hc * BC:(hc + 1) * BC],
                               ASPT[:sz, hc * BC:(hc + 1) * BC])
            LG = sm_ps.tile([BC, 3], F32, tag="sm")
            for hc, (hlo, hhi) in enumerate(HCH):
                sz = hhi - hlo
                nc.tensor.matmul(
                    LG[:], ASB[:sz, hc * BC:(hc + 1) * BC], CLSW[:sz, hc, :],
                    start=(hc == 0), stop=(hc == len(HCH) - 1))
            OSB = stats.tile([BC, 3], F32, tag="osb")
            nc.vector.tensor_scalar(OSB[:], LG[:], RECIP[:], None, ALU.mult)
            nc.vector.tensor_add(OSB[:], OSB[:], CLSB[:])
            nc.sync.dma_start(out_ap[:], OSB[:])

        if repeats == 1:
            body()
        else:
            with tc.For_i(0, repeats, 1):
                body()

    nc.compile()
    return nc


def host_inputs(inputs):
    """Slice/prepare per-core input maps from the full problem inputs."""
    hs12 = np.ascontiguousarray(np.asarray(inputs["hidden_states"])[12])  # [B,L,D]
    ts = np.asarray(inputs["token_starts"]).astype(np.float32)
    m = np.ascontiguousarray(np.asarray(inputs["aspect_in_text_mask"], dtype=np.float32))
    gw = np.ascontiguousarray(np.asarray(inputs["guid_W"], dtype=np.float32)[3])
    gb = np.asarray(inputs["guid_b"], dtype=np.float32)[3]
    ln_g = np.asarray(inputs["ln_g"], dtype=np.float32)
    ln_b = np.asarray(inputs["ln_b"], dtype=np.float32)
    cls_W = np.asarray(inputs["cls_W"], dtype=np.float32)
    cls_b = np.asarray(inputs["cls_b"], dtype=np.float32)

    gbrep = np.tile(gb[None, :], (128, 1)).astype(np.float32)
    clsw_eff = (ln_g[:, None] * cls_W).astype(np.float32)
    clsw_pad = np.zeros((640, 3), np.float32)
    clsw_pad[:H] = clsw_eff
    clsb_eff = (ln_b @ cls_W + cls_b).astype(np.float32)
    clsb_rep = np.tile(clsb_eff[None, :], (BC, 1)).astype(np.float32)
    iota = np.tile(np.arange(L, dtype=np.float32)[None, :], (128, 1))
    eye = np.eye(128, dtype=np.float32)

    in_maps = []
    for c in range(N_CORES):
        sl = slice(c * BC, (c + 1) * BC)
        in_maps.append(dict(
            hs=np.ascontiguousarray(hs12[sl]),
            gw=gw,
            gbrep=gbrep,
            tsf=np.ascontiguousarray(ts[sl].T),
            mT=np.ascontiguousarray(m[sl].T),
            mnat=np.ascontiguousarray(m[sl]),
            iota=iota,
            eye=eye,
            clsw=clsw_pad,
            clsb=clsb_rep,
        ))
    return in_maps


_PROGRAM = None


def kernel(**inputs):
    global _PROGRAM
    if _PROGRAM is None:
        _PROGRAM = build_program(repeats=1)
    nc = _PROGRAM
    in_maps = host_inputs(inputs)
    res = run_bass_kernel_spmd(nc, in_maps, list(range(N_CORES)), trace=False)
    out = np.concatenate([res.results[c]["out"] for c in range(N_CORES)], axis=0)
    return out.astype(np.float32)


# revision 12
# speedup vs baseline: 1.2787x; 1.2787x over previous
"""Trainium2 Bass kernel for nn_BERT4GCN_53884659695997.

Mathematical reduction
----------------------
In the reference, ``feature`` is reassigned to ``LN(guidance)`` at the top of
every loop iteration, so the GCN block's output is never consumed; only the
last BERT layer's branch (index 3 -> hidden_states layer 12, which skips the
GCN block) reaches the output:

    t[b]      = LN(relu(hs[12,b][ts[b]] @ guid_W[3] + guid_b[3])) * ln_g + ln_b
    logits[b] = ((t[b] * m[b,:,None]).sum(0) / m[b].sum()) @ cls_W + cls_b

(verified numerically against the jax reference to ~7e-7 rel err).

Row gathers commute with the row-wise ops (matmul-by-row / relu / LN), so we
compute on the 256 *source* rows and fold gather+mask into per-source-row
weights  w[r] = sum_i m[i] * [ts[i] == r],  built on device via a one-hot
matmul.  The LN affine output is never materialized: with per-row stats
(mu, rs) and w2 = w * rs,

    sum_r w[r] * (GR[r,:] - mu[r]) * rs[r] = GR^T @ w2 - (mu . w2) * ones

so normalization folds into the aspect reduction (PE) plus a scalar
correction.  ln_g / ln_b are folded into cls_W / cls_b host-side and
guid_b enters the guidance matmul as a K=1 ones-row term (exact fp32
linear algebra either way).

Sharding: data-parallel over batch B=64 -> 8 samples per core on 8 cores.
Main matmuls run as float32r (4-byte operands, full-rate streaming for
moving dims >= 256); reductions accumulate in fp32 PSUM.
"""

import numpy as np
from contextlib import ExitStack

import concourse.bass as bass
import concourse.tile as tile
from concourse import bacc, mybir
from concourse.bass_utils import run_bass_kernel_spmd

F32 = mybir.dt.float32
F32R = mybir.dt.float32r
AX = mybir.AxisListType
ALU = mybir.AluOpType
ACTF = mybir.ActivationFunctionType

N_CORES = 8
B = 64
BC = B // N_CORES
L = 256
D = 768
H = 600
EPS = 1e-5
KT = D // 128    # 6 k-tiles
IT = L // 128    # 2 row-tiles
NCH = ((0, 344), (344, 600))    # both chunks >= 256 for float32r full rate
HCH = ((0, 128), (128, 256), (256, 384), (384, 512), (512, 600))


def r(ap):
    return ap.bitcast(F32R)


def build_program(repeats: int = 1):
    nc = bacc.Bacc("TRN2", target_bir_lowering=False, debug=False,
                   num_devices=N_CORES)

    dr = {}
    def din(name, shape):
        dr[name] = nc.dram_tensor(name, list(shape), F32, kind="ExternalInput").ap()
    din("hs", (BC, L, D))
    din("gw", (D, H))
    din("gbrow", (1, H))
    din("onesrow", (1, 128))
    din("tsf", (L, BC))
    din("mT", (L, BC))
    din("mnat", (BC, L))
    din("iota", (128, L))
    din("eye", (128, 128))
    din("clsw", (640, 3))      # ln_g-folded cls_W, zero-padded 600->640
    din("clsb", (BC, 3))       # ln_b@cls_W + cls_b, replicated rows
    din("srep", (BC, 3))       # column sums of folded cls_W, replicated rows
    out_ap = nc.dram_tensor("out", [BC, 3], F32, kind="ExternalOutput").ap()

    with tile.TileContext(nc) as tc, ExitStack() as ctx:
        cpool = ctx.enter_context(tc.tile_pool(name="consts", bufs=1))
        hpool = ctx.enter_context(tc.tile_pool(name="hs", bufs=2))
        tpool = ctx.enter_context(tc.tile_pool(name="hst", bufs=2))
        apool = ctx.enter_context(tc.tile_pool(name="act", bufs=2))
        spool = ctx.enter_context(tc.tile_pool(name="small", bufs=2))
        stats = ctx.enter_context(tc.tile_pool(name="stats", bufs=1))
        pg_ps = ctx.enter_context(tc.tile_pool(name="pg", bufs=4, space="PSUM"))
        sm_ps = ctx.enter_context(tc.tile_pool(name="sm", bufs=3, space="PSUM"))
        asp_ps = ctx.enter_context(tc.tile_pool(name="asp", bufs=1, space="PSUM"))

        # ---- constants (loaded once) ----
        GW0 = cpool.tile([128, KT, H], F32, tag="gw0")
        nc.sync.dma_start(GW0[:], dr["gw"].rearrange("(k p) n -> p k n", p=128))
        GW = cpool.tile([128, KT, H], F32R, tag="gw")
        nc.vector.tensor_copy(GW[:], GW0[:])
        GBROW0 = cpool.tile([1, H], F32, tag="gbrow0")
        nc.sync.dma_start(GBROW0[:], dr["gbrow"][:])
        GBROW = cpool.tile([1, H], F32R, tag="gbrow")
        nc.vector.tensor_copy(GBROW[:], GBROW0[:])
        ONESR0 = cpool.tile([1, 128], F32, tag="onesrow0")
        nc.sync.dma_start(ONESR0[:], dr["onesrow"][:])
        ONESR = cpool.tile([1, 128], F32R, tag="onesrow")
        nc.vector.tensor_copy(ONESR[:], ONESR0[:])
        IOTA = cpool.tile([128, L], F32, tag="iota")
        nc.sync.dma_start(IOTA[:], dr["iota"][:])
        EYE = cpool.tile([128, 128], F32, tag="eye")
        nc.sync.dma_start(EYE[:], dr["eye"][:])
        TSF = cpool.tile([128, IT, BC], F32, tag="tsf")
        nc.sync.dma_start(TSF[:], dr["tsf"].rearrange("(t p) s -> p t s", p=128))
        MT = cpool.tile([128, IT, BC], F32, tag="mt")
        nc.sync.dma_start(MT[:], dr["mT"].rearrange("(t p) s -> p t s", p=128))
        MN = cpool.tile([BC, L], F32, tag="mn")
        nc.sync.dma_start(MN[:], dr["mnat"][:])
        CLSW = cpool.tile([128, 5, 3], F32, tag="clsw")
        nc.sync.dma_start(CLSW[:], dr["clsw"].rearrange("(c p) n -> p c n", p=128))
        CLSB = cpool.tile([BC, 3], F32, tag="clsb")
        nc.sync.dma_start(CLSB[:], dr["clsb"][:])
        SREP = cpool.tile([BC, 3], F32, tag="srep")
        nc.sync.dma_start(SREP[:], dr["srep"][:])

        # 1/sum(m) per sample
        SM = stats.tile([BC, 1], F32, tag="sm")
        nc.vector.tensor_reduce(SM[:], MN[:], AX.X, ALU.add)
        RECIP = stats.tile([BC, 1], F32, tag="recip")
        nc.vector.reciprocal(RECIP[:], SM[:])

        # LN stats accumulators, one column per (sample, row-tile)
        S1A = stats.tile([128, 2 * BC], F32, tag="s1a")
        S1B = stats.tile([128, 2 * BC], F32, tag="s1b")
        S2 = stats.tile([128, 2 * BC], F32, tag="s2")
        MU = stats.tile([128, 2 * BC], F32, tag="mu")
        RS = stats.tile([128, 2 * BC], F32, tag="rs")

        def body():
            ASPT = asp_ps.tile([128, 5 * BC], F32, tag="aspt")
            CPS = sm_ps.tile([1, BC], F32, tag="sm")
            for s in range(BC):
                # -------- load sample, transpose to [d, i] layout --------
                HSN = hpool.tile([128, IT, D], F32, tag="hsn")
                nc.sync.dma_start(HSN[:], dr["hs"][s].rearrange("(t p) d -> p t d", p=128))
                HST = tpool.tile([128, KT, L], F32R, tag="hst")
                for kt in range(KT):
                    for it in range(IT):
                        PT = sm_ps.tile([128, 128], F32, tag="sm")
                        nc.tensor.transpose(PT[:], HSN[:, it, kt * 128:(kt + 1) * 128], EYE[:])
                        nc.vector.tensor_copy(HST[:, kt, it * 128:(it + 1) * 128], PT[:])

                # -------- guidance matmul (float32r) + relu + stats --------
                GR2 = apool.tile([128, IT, H], F32, tag="gr2")
                for mt in range(IT):
                    col = s * 2 + mt
                    for ci, (nlo, nhi) in enumerate(NCH):
                        PG = pg_ps.tile([128, nhi - nlo], F32, tag="pg")
                        for kt in range(KT):
                            nc.tensor.matmul(
                                PG[:], HST[:, kt, mt * 128:(mt + 1) * 128],
                                GW[:, kt, nlo:nhi],
                                start=(kt == 0), stop=False)
                        nc.tensor.matmul(
                            PG[:], ONESR[:],
                            GBROW[:, nlo:nhi], start=False, stop=True)
                        acc = (S1A if ci == 0 else S1B)[:, col:col + 1]
                        nc.scalar.activation(GR2[:, mt, nlo:nhi], PG[:], ACTF.Relu,
                                             accum_out=acc)
                    SQ = apool.tile([128, H], F32, tag="sq")
                    nc.scalar.activation(SQ[:], GR2[:, mt, :], ACTF.Square,
                                         accum_out=S2[:, col:col + 1])
                    c1 = slice(col, col + 1)
                    nc.vector.tensor_add(MU[:, c1], S1A[:, c1], S1B[:, c1])
                    nc.vector.tensor_scalar_mul(MU[:, c1], MU[:, c1], 1.0 / H)
                    V = spool.tile([128, 1], F32, tag="v")
                    nc.vector.tensor_scalar_mul(V[:], S2[:, c1], 1.0 / H)
                    MSQ = spool.tile([128, 1], F32, tag="msq")
                    nc.vector.tensor_mul(MSQ[:], MU[:, c1], MU[:, c1])
                    nc.vector.tensor_sub(V[:], V[:], MSQ[:])
                    nc.vector.tensor_scalar_add(V[:], V[:], EPS)
                    SD = spool.tile([128, 1], F32, tag="sd")
                    nc.scalar.sqrt(SD[:], V[:])
                    nc.vector.reciprocal(RS[:, c1], SD[:])

                # -------- gather weights w[r] = sum_i m[i][ts[i]==r] --------
                WPS = sm_ps.tile([128, IT], F32, tag="sm")
                SOHs = []
                for it in range(IT):
                    SOH = spool.tile([128, L], F32, tag="soh")
                    nc.vector.tensor_scalar(SOH[:], IOTA[:], TSF[:, it, s:s + 1], None,
                                            ALU.is_equal)
                    SOHs.append(SOH)
                for rt in range(IT):
                    for it in range(IT):
                        nc.tensor.matmul(
                            WPS[:, rt:rt + 1], SOHs[it][:, rt * 128:(rt + 1) * 128],
                            MT[:, it, s:s + 1],
                            start=(it == 0), stop=(it == IT - 1))
                # w2 = w * rstd (folds LN scale into the reduction weights)
                W2 = spool.tile([128, IT], F32, tag="w2")
                nc.vector.tensor_mul(W2[:], WPS[:], RS[:, s * 2:s * 2 + 2])

                # -------- aspects^T column s + mean correction ------------
                for hc, (hlo, hhi) in enumerate(HCH):
                    for it in range(IT):
                        nc.tensor.matmul(
                            ASPT[:hhi - hlo, hc * BC + s:hc * BC + s + 1],
                            GR2[:, it, hlo:hhi], W2[:, it:it + 1],
                            start=(it == 0), stop=(it == IT - 1))
                for it in range(IT):
                    nc.tensor.matmul(
                        CPS[:, s:s + 1], MU[:, s * 2 + it:s * 2 + it + 1],
                        W2[:, it:it + 1],
                        start=(it == 0), stop=(it == IT - 1))

            # -------- classifier --------
            ASB = stats.tile([128, 5 * BC], F32, tag="asb")
            for hc, (hlo, hhi) in enumerate(HCH):
                sz = hhi - hlo
                nc.scalar.copy(ASB[:sz, hc * BC:(hc + 1) * BC],
                               ASPT[:sz, hc * BC:(hc + 1) * BC])
            CROW = stats.tile([1, BC], F32, tag="crow")
            nc.vector.tensor_copy(CROW[:], CPS[:])
            CTP = sm_ps.tile([BC, 1], F32, tag="sm")
            nc.tensor.transpose(CTP[:], CROW[:], EYE[0:1, 0:1])
            CT = stats.tile([BC, 1], F32, tag="ct")
            nc.vector.tensor_copy(CT[:], CTP[:])

            LG = sm_ps.tile([BC, 3], F32, tag="sm")
            for hc, (hlo, hhi) in enumerate(HCH):
                sz = hhi - hlo
                nc.tensor.matmul(
                    LG[:], ASB[:sz, hc * BC:(hc + 1) * BC], CLSW[:sz, hc, :],
                    start=(hc == 0), stop=(hc == len(HCH) - 1))
            T1 = stats.tile([BC, 3], F32, tag="t1")
            nc.vector.tensor_scalar(T1[:], SREP[:], CT[:], None, ALU.mult)
            OSB = stats.tile([BC, 3], F32, tag="osb")
            nc.vector.tensor_sub(OSB[:], LG[:], T1[:])
            nc.vector.tensor_scalar(OSB[:], OSB[:], RECIP[:], None, ALU.mult)
            nc.vector.tensor_add(OSB[:], OSB[:], CLSB[:])
            nc.sync.dma_start(out_ap[:], OSB[:])

        if repeats == 1:
            body()
        else:
            with tc.For_i(0, repeats, 1):
                body()

    nc.compile()
    return nc


def host_inputs(inputs):
    """Slice/prepare per-core input maps from the full problem inputs."""
    hs12 = np.ascontiguousarray(np.asarray(inputs["hidden_states"])[12])  # [B,L,D]
    ts = np.asarray(inputs["token_starts"]).astype(np.float32)
    m = np.ascontiguousarray(np.asarray(inputs["aspect_in_text_mask"], dtype=np.float32))
    gw = np.ascontiguousarray(np.asarray(inputs["guid_W"], dtype=np.float32)[3])
    gb = np.asarray(inputs["guid_b"], dtype=np.float32)[3]
    ln_g = np.asarray(inputs["ln_g"], dtype=np.float32)
    ln_b = np.asarray(inputs["ln_b"], dtype=np.float32)
    cls_W = np.asarray(inputs["cls_W"], dtype=np.float32)
    cls_b = np.asarray(inputs["cls_b"], dtype=np.float32)

    clsw_eff = (ln_g[:, None] * cls_W).astype(np.float32)
    clsw_pad = np.zeros((640, 3), np.float32)
    clsw_pad[:H] = clsw_eff
    clsb_eff = (ln_b @ cls_W + cls_b).astype(np.float32)
    clsb_rep = np.tile(clsb_eff[None, :], (BC, 1)).astype(np.float32)
    srep = np.tile(clsw_eff.sum(0, dtype=np.float32)[None, :], (BC, 1)).astype(np.float32)
    iota = np.tile(np.arange(L, dtype=np.float32)[None, :], (128, 1))
    eye = np.eye(128, dtype=np.float32)
    onesrow = np.ones((1, 128), np.float32)

    in_maps = []
    for c in range(N_CORES):
        sl = slice(c * BC, (c + 1) * BC)
        in_maps.append(dict(
            hs=np.ascontiguousarray(hs12[sl]),
            gw=gw,
            gbrow=gb[None, :],
            onesrow=onesrow,
            tsf=np.ascontiguousarray(ts[sl].T),
            mT=np.ascontiguousarray(m[sl].T),
            mnat=np.ascontiguousarray(m[sl]),
            iota=iota,
            eye=eye,
            clsw=clsw_pad,
            clsb=clsb_rep,
            srep=srep,
        ))
    return in_maps


_PROGRAM = None


def kernel(**inputs):
    global _PROGRAM
    if _PROGRAM is None:
        _PROGRAM = build_program(repeats=1)
    nc = _PROGRAM
    in_maps = host_inputs(inputs)
    res = run_bass_kernel_spmd(nc, in_maps, list(range(N_CORES)), trace=False)
    out = np.concatenate([res.results[c]["out"] for c in range(N_CORES)], axis=0)
    return out.astype(np.float32)


# revision 18
# speedup vs baseline: 1.3049x; 1.0204x over previous
"""Trainium2 Bass kernel for nn_BERT4GCN_53884659695997.

Mathematical reduction
----------------------
In the reference, ``feature`` is reassigned to ``LN(guidance)`` at the top of
every loop iteration, so the GCN block's output is never consumed; only the
last BERT layer's branch (index 3 -> hidden_states layer 12, which skips the
GCN block) reaches the output:

    t[b]      = LN(relu(hs[12,b][ts[b]] @ guid_W[3] + guid_b[3])) * ln_g + ln_b
    logits[b] = ((t[b] * m[b,:,None]).sum(0) / m[b].sum()) @ cls_W + cls_b

(verified numerically against the jax reference to ~7e-7 rel err).

Row gathers commute with the row-wise ops (matmul-by-row / relu / LN), so we
compute on the 256 *source* rows and fold gather+mask into per-source-row
weights  w[r] = sum_i m[i] * [ts[i] == r],  built on device via a one-hot
matmul.  The LN affine output is never materialized: with per-row stats
(mu, rs) and w2 = w * rs,

    sum_r w[r] * (GR[r,:] - mu[r]) * rs[r] = GR^T @ w2 - (mu . w2) * ones

so normalization folds into the aspect reduction (PE) plus a scalar
correction.  ln_g / ln_b are folded into cls_W / cls_b host-side and
guid_b enters the guidance matmul as a K=1 ones-row term (exact fp32
linear algebra either way).

Sharding: data-parallel over batch B=64 -> 8 samples per core on 8 cores.
Main matmuls run as float32r (4-byte operands, full-rate streaming for
moving dims >= 256); reductions accumulate in fp32 PSUM.
"""

import numpy as np
from contextlib import ExitStack

import concourse.bass as bass
import concourse.tile as tile
from concourse import bacc, mybir
from concourse.bass_utils import run_bass_kernel_spmd

F32 = mybir.dt.float32
F32R = mybir.dt.float32r
AX = mybir.AxisListType
ALU = mybir.AluOpType
ACTF = mybir.ActivationFunctionType

N_CORES = 8
B = 64
BC = B // N_CORES
L = 256
D = 768
H = 600
EPS = 1e-5
KT = D // 128    # 6 k-tiles
IT = L // 128    # 2 row-tiles
NCH = ((0, 344), (344, 600))    # both chunks >= 256 for float32r full rate
HCH = ((0, 128), (128, 256), (256, 384), (384, 512), (512, 600))


def r(ap):
    return ap.bitcast(F32R)


def build_program(repeats: int = 1):
    nc = bacc.Bacc("TRN2", target_bir_lowering=False, debug=False,
                   num_devices=N_CORES)

    dr = {}
    def din(name, shape):
        dr[name] = nc.dram_tensor(name, list(shape), F32, kind="ExternalInput").ap()
    din("hs", (BC, L, D))
    din("gw", (D, H))
    din("gbrow", (1, H))
    din("onesrow", (1, 128))
    din("tsf", (L, BC))
    din("mT", (L, BC))
    din("mnat", (BC, L))
    din("iota", (128, L))
    din("eye", (128, 128))
    din("clsw", (640, 3))      # ln_g-folded cls_W, zero-padded 600->640
    din("clsb", (BC, 3))       # ln_b@cls_W + cls_b, replicated rows
    din("srep", (BC, 3))       # column sums of folded cls_W, replicated rows
    out_ap = nc.dram_tensor("out", [BC, 3], F32, kind="ExternalOutput").ap()

    with tile.TileContext(nc) as tc, ExitStack() as ctx:
        cpool = ctx.enter_context(tc.tile_pool(name="consts", bufs=1))
        hpool = ctx.enter_context(tc.tile_pool(name="hs", bufs=2))
        tpool = ctx.enter_context(tc.tile_pool(name="hst", bufs=2))
        apool = ctx.enter_context(tc.tile_pool(name="act", bufs=2))
        spool = ctx.enter_context(tc.tile_pool(name="small", bufs=2))
        stats = ctx.enter_context(tc.tile_pool(name="stats", bufs=1))
        pg_ps = ctx.enter_context(tc.tile_pool(name="pg", bufs=4, space="PSUM"))
        sm_ps = ctx.enter_context(tc.tile_pool(name="sm", bufs=2, space="PSUM"))
        asp_ps = ctx.enter_context(tc.tile_pool(name="asp", bufs=1, space="PSUM"))

        # ---- constants (loaded once) ----
        GW0 = cpool.tile([128, KT, H], F32, tag="gw0")
        nc.sync.dma_start(GW0[:], dr["gw"].rearrange("(k p) n -> p k n", p=128))
        GW = cpool.tile([128, KT, H], F32R, tag="gw")
        nc.vector.tensor_copy(GW[:], GW0[:])
        GBROW0 = cpool.tile([1, H], F32, tag="gbrow0")
        nc.sync.dma_start(GBROW0[:], dr["gbrow"][:])
        GBROW = cpool.tile([1, H], F32R, tag="gbrow")
        nc.vector.tensor_copy(GBROW[:], GBROW0[:])
        ONESR0 = cpool.tile([1, 128], F32, tag="onesrow0")
        nc.sync.dma_start(ONESR0[:], dr["onesrow"][:])
        ONESR = cpool.tile([1, 128], F32R, tag="onesrow")
        nc.vector.tensor_copy(ONESR[:], ONESR0[:])
        IOTA = cpool.tile([128, L], F32, tag="iota")
        nc.sync.dma_start(IOTA[:], dr["iota"][:])
        EYE = cpool.tile([128, 128], F32, tag="eye")
        nc.sync.dma_start(EYE[:], dr["eye"][:])
        TSF = cpool.tile([128, IT, BC], F32, tag="tsf")
        nc.sync.dma_start(TSF[:], dr["tsf"].rearrange("(t p) s -> p t s", p=128))
        MT = cpool.tile([128, IT, BC], F32, tag="mt")
        nc.sync.dma_start(MT[:], dr["mT"].rearrange("(t p) s -> p t s", p=128))
        MN = cpool.tile([BC, L], F32, tag="mn")
        nc.sync.dma_start(MN[:], dr["mnat"][:])
        CLSW = cpool.tile([128, 5, 3], F32, tag="clsw")
        nc.sync.dma_start(CLSW[:], dr["clsw"].rearrange("(c p) n -> p c n", p=128))
        CLSB = cpool.tile([BC, 3], F32, tag="clsb")
        nc.sync.dma_start(CLSB[:], dr["clsb"][:])
        SREP = cpool.tile([BC, 3], F32, tag="srep")
        nc.sync.dma_start(SREP[:], dr["srep"][:])

        # 1/sum(m) per sample
        SM = stats.tile([BC, 1], F32, tag="sm")
        nc.vector.tensor_reduce(SM[:], MN[:], AX.X, ALU.add)
        RECIP = stats.tile([BC, 1], F32, tag="recip")
        nc.vector.reciprocal(RECIP[:], SM[:])

        # LN stats accumulators, one column per (sample, row-tile)
        S1A = stats.tile([128, 2 * BC], F32, tag="s1a")
        S1B = stats.tile([128, 2 * BC], F32, tag="s1b")
        S2 = stats.tile([128, 2 * BC], F32, tag="s2")
        MU = stats.tile([128, 2 * BC], F32, tag="mu")
        RS = stats.tile([128, 2 * BC], F32, tag="rs")

        def body():
            ASPT = asp_ps.tile([128, 5 * BC], F32, tag="aspt")
            CPS = sm_ps.tile([1, BC], F32, tag="cps")
            for s in range(BC):
                # -------- load sample, transpose to [d, i] layout --------
                HSN = hpool.tile([128, IT, D], F32, tag="hsn")
                nc.sync.dma_start(HSN[:], dr["hs"][s].rearrange("(t p) d -> p t d", p=128))
                HST = tpool.tile([128, KT, L], F32R, tag="hst")
                for kt in range(KT):
                    for it in range(IT):
                        PT = pg_ps.tile([128, 128], F32, tag="pg")
                        nc.tensor.transpose(PT[:], HSN[:, it, kt * 128:(kt + 1) * 128], EYE[:])
                        nc.vector.tensor_copy(HST[:, kt, it * 128:(it + 1) * 128], PT[:])

                # -------- guidance matmul (float32r) + relu + stats --------
                GR2 = apool.tile([128, IT, H], F32, tag="gr2")
                for mt in range(IT):
                    col = s * 2 + mt
                    for ci, (nlo, nhi) in enumerate(NCH):
                        PG = pg_ps.tile([128, nhi - nlo], F32, tag="pg")
                        for kt in range(KT):
                            nc.tensor.matmul(
                                PG[:], HST[:, kt, mt * 128:(mt + 1) * 128],
                                GW[:, kt, nlo:nhi],
                                start=(kt == 0), stop=False)
                        nc.tensor.matmul(
                            PG[:], ONESR[:],
                            GBROW[:, nlo:nhi], start=False, stop=True)
                        acc = (S1A if ci == 0 else S1B)[:, col:col + 1]
                        nc.scalar.activation(GR2[:, mt, nlo:nhi], PG[:], ACTF.Relu,
                                             accum_out=acc)
                    SQ = apool.tile([128, H], F32, tag="sq")
                    nc.scalar.activation(SQ[:], GR2[:, mt, :], ACTF.Square,
                                         accum_out=S2[:, col:col + 1])
                    c1 = slice(col, col + 1)
                    nc.vector.tensor_add(MU[:, c1], S1A[:, c1], S1B[:, c1])
                    nc.vector.tensor_scalar_mul(MU[:, c1], MU[:, c1], 1.0 / H)
                    V = spool.tile([128, 1], F32, tag="v")
                    nc.vector.tensor_scalar_mul(V[:], S2[:, c1], 1.0 / H)
                    MSQ = spool.tile([128, 1], F32, tag="msq")
                    nc.vector.tensor_mul(MSQ[:], MU[:, c1], MU[:, c1])
                    nc.vector.tensor_sub(V[:], V[:], MSQ[:])
                    nc.vector.tensor_scalar_add(V[:], V[:], EPS)
                    SD = spool.tile([128, 1], F32, tag="sd")
                    nc.scalar.sqrt(SD[:], V[:])
                    nc.vector.reciprocal(RS[:, c1], SD[:])

                # -------- gather weights w[r] = sum_i m[i][ts[i]==r] --------
                WPS = sm_ps.tile([128, IT], F32, tag="cps")
                SOHs = []
                for it in range(IT):
                    SOH = spool.tile([128, L], F32, tag="soh")
                    nc.vector.tensor_scalar(SOH[:], IOTA[:], TSF[:, it, s:s + 1], None,
                                            ALU.is_equal)
                    SOHs.append(SOH)
                for rt in range(IT):
                    for it in range(IT):
                        nc.tensor.matmul(
                            WPS[:, rt:rt + 1], SOHs[it][:, rt * 128:(rt + 1) * 128],
                            MT[:, it, s:s + 1],
                            start=(it == 0), stop=(it == IT - 1))
                # w2 = w * rstd (folds LN scale into the reduction weights)
                W2 = spool.tile([128, IT], F32, tag="w2")
                nc.vector.tensor_mul(W2[:], WPS[:], RS[:, s * 2:s * 2 + 2])

                # -------- aspects^T column s + mean correction ------------
                for hc, (hlo, hhi) in enumerate(HCH):
                    for it in range(IT):
                        nc.tensor.matmul(
                            ASPT[:hhi - hlo, hc * BC + s:hc * BC + s + 1],
                            GR2[:, it, hlo:hhi], W2[:, it:it + 1],
                            start=(it == 0), stop=(it == IT - 1))
                for it in range(IT):
                    nc.tensor.matmul(
                        CPS[:, s:s + 1], MU[:, s * 2 + it:s * 2 + it + 1],
                        W2[:, it:it + 1],
                        start=(it == 0), stop=(it == IT - 1))

            # -------- classifier --------
            ASB = stats.tile([128, 5 * BC], F32, tag="asb")
            for hc, (hlo, hhi) in enumerate(HCH):
                sz = hhi - hlo
                nc.scalar.copy(ASB[:sz, hc * BC:(hc + 1) * BC],
                               ASPT[:sz, hc * BC:(hc + 1) * BC])
            CROW = stats.tile([1, BC], F32, tag="crow")
            nc.vector.tensor_copy(CROW[:], CPS[:])
            CTP = sm_ps.tile([BC, 1], F32, tag="cps")
            nc.tensor.transpose(CTP[:], CROW[:], EYE[0:1, 0:1])
            CT = stats.tile([BC, 1], F32, tag="ct")
            nc.vector.tensor_copy(CT[:], CTP[:])

            LG = sm_ps.tile([BC, 3], F32, tag="cps")
            for hc, (hlo, hhi) in enumerate(HCH):
                sz = hhi - hlo
                nc.tensor.matmul(
                    LG[:], ASB[:sz, hc * BC:(hc + 1) * BC], CLSW[:sz, hc, :],
                    start=(hc == 0), stop=(hc == len(HCH) - 1))
            T1 = stats.tile([BC, 3], F32, tag="t1")
            nc.vector.tensor_scalar(T1[:], SREP[:], CT[:], None, ALU.mult)
            OSB = stats.tile([BC, 3], F32, tag="osb")
            nc.vector.tensor_sub(OSB[:], LG[:], T1[:])
            nc.vector.tensor_scalar(OSB[:], OSB[:], RECIP[:], None, ALU.mult)
            nc.vector.tensor_add(OSB[:], OSB[:], CLSB[:])
            nc.sync.dma_start(out_ap[:], OSB[:])

        if repeats == 1:
            body()
        else:
            with tc.For_i(0, repeats, 1):
                body()

    nc.compile()
    return nc


def host_inputs(inputs):
    """Slice/prepare per-core input maps from the full problem inputs."""
    hs12 = np.ascontiguousarray(np.asarray(inputs["hidden_states"])[12])  # [B,L,D]
    ts = np.asarray(inputs["token_starts"]).astype(np.float32)
    m = np.ascontiguousarray(np.asarray(inputs["aspect_in_text_mask"], dtype=np.float32))
    gw = np.ascontiguousarray(np.asarray(inputs["guid_W"], dtype=np.float32)[3])
    gb = np.asarray(inputs["guid_b"], dtype=np.float32)[3]
    ln_g = np.asarray(inputs["ln_g"], dtype=np.float32)
    ln_b = np.asarray(inputs["ln_b"], dtype=np.float32)
    cls_W = np.asarray(inputs["cls_W"], dtype=np.float32)
    cls_b = np.asarray(inputs["cls_b"], dtype=np.float32)

    clsw_eff = (ln_g[:, None] * cls_W).astype(np.float32)
    clsw_pad = np.zeros((640, 3), np.float32)
    clsw_pad[:H] = clsw_eff
    clsb_eff = (ln_b @ cls_W + cls_b).astype(np.float32)
    clsb_rep = np.tile(clsb_eff[None, :], (BC, 1)).astype(np.float32)
    srep = np.tile(clsw_eff.sum(0, dtype=np.float32)[None, :], (BC, 1)).astype(np.float32)
    iota = np.tile(np.arange(L, dtype=np.float32)[None, :], (128, 1))
    eye = np.eye(128, dtype=np.float32)
    onesrow = np.ones((1, 128), np.float32)

    in_maps = []
    for c in range(N_CORES):
        sl = slice(c * BC, (c + 1) * BC)
        in_maps.append(dict(
            hs=np.ascontiguousarray(hs12[sl]),
            gw=gw,
            gbrow=gb[None, :],
            onesrow=onesrow,
            tsf=np.ascontiguousarray(ts[sl].T),
            mT=np.ascontiguousarray(m[sl].T),
            mnat=np.ascontiguousarray(m[sl]),
            iota=iota,
            eye=eye,
            clsw=clsw_pad,
            clsb=clsb_rep,
            srep=srep,
        ))
    return in_maps


_PROGRAM = None


def kernel(**inputs):
    global _PROGRAM
    if _PROGRAM is None:
        _PROGRAM = build_program(repeats=1)
    nc = _PROGRAM
    in_maps = host_inputs(inputs)
    res = run_bass_kernel_spmd(nc, in_maps, list(range(N_CORES)), trace=False)
    out = np.concatenate([res.results[c]["out"] for c in range(N_CORES)], axis=0)
    return out.astype(np.float32)


# revision 20
# speedup vs baseline: 2.5065x; 1.9209x over previous
"""Trainium2 Bass kernel for nn_BERT4GCN_53884659695997.

Mathematical reduction
----------------------
In the reference, ``feature`` is reassigned to ``LN(guidance)`` at the top of
every loop iteration, so the GCN block's output is never consumed; only the
last BERT layer's branch (index 3 -> hidden_states layer 12, which skips the
GCN block) reaches the output:

    t[b]      = LN(relu(hs[12,b][ts[b]] @ guid_W[3] + guid_b[3])) * ln_g + ln_b
    logits[b] = ((t[b] * m[b,:,None]).sum(0) / m[b].sum(0)) @ cls_W + cls_b

(verified numerically against the jax reference to ~7e-7 rel err).

Row gathers commute with the row-wise ops (matmul-by-row / relu / LN), so the
gather+mask folds into per-source-row weights w[r] = sum_i m[i]*[ts[i]==r].
Only rows with w[r] != 0 can reach the output, and there are at most
|unique(ts[b][m[b]>0])| ~ 51 of them per sample, so each sample's work is
compacted to K=128 rows: the host emits the compact row list (pure index
bookkeeping; all tensor arithmetic stays on device), and the device gathers
those rows *inside* the layout-transpose matmul (in^T @ G with a one-hot G
instead of the identity).  LN is per-row, so compaction is exact.

The LN affine output is never materialized: with per-row stats (mu, rs) and
w2 = w * rs,

    sum_r w[r] * (GR[r,:] - mu[r]) * rs[r] = GR^T @ w2 - (mu . w2) * ones

so normalization folds into the aspect reduction (PE) plus a scalar
correction.  ln_g / ln_b fold into cls_W / cls_b host-side and guid_b enters
the guidance matmul as a K=1 ones-row term (exact fp32 linear algebra).

Sharding: data-parallel over batch B=64 -> 8 samples per core on 8 cores.
The guidance matmul runs as float32r (4-byte operands, full-rate streaming
for moving dims >= 256); reductions accumulate in fp32 PSUM.
"""

import numpy as np
from contextlib import ExitStack

import concourse.bass as bass
import concourse.tile as tile
from concourse import bacc, mybir
from concourse.bass_utils import run_bass_kernel_spmd

F32 = mybir.dt.float32
F32R = mybir.dt.float32r
AX = mybir.AxisListType
ALU = mybir.AluOpType
ACTF = mybir.ActivationFunctionType

N_CORES = 8
B = 64
BC = B // N_CORES
L = 256
D = 768
H = 600
KC = 128        # compact row budget per sample (unique masked starts ~51)
EPS = 1e-5
KT = D // 128   # 6 k-tiles
IT = L // 128   # 2 source-row tiles
NCH = ((0, 344), (344, 600))   # both chunks >= 256 for float32r full rate
HCH = ((0, 128), (128, 256), (256, 384), (384, 512), (512, 600))


def build_program(repeats: int = 1):
    nc = bacc.Bacc("TRN2", target_bir_lowering=False, debug=False,
                   num_devices=N_CORES)

    dr = {}
    def din(name, shape, dt=F32):
        dr[name] = nc.dram_tensor(name, list(shape), dt, kind="ExternalInput").ap()
    din("hs", (BC, L, D))
    din("gw", (D, H))
    din("gbrow", (1, H))
    din("onesrow", (1, 128))
    din("rows", (1, BC * KC))     # compact row values per sample (0..255)
    din("pidx2", (128, IT))       # [p, p+128]
    din("tscT", (L, BC))          # compact index of ts[i], masked-only
    din("mT", (L, BC))
    din("mnat", (BC, L))
    din("iota", (128, KC))
    din("eye", (128, 128))
    din("clsw", (640, 3))         # ln_g-folded cls_W, zero-padded 600->640
    din("clsb", (BC, 3))          # ln_b@cls_W + cls_b, replicated rows
    din("srep", (BC, 3))          # column sums of folded cls_W, replicated
    out_ap = nc.dram_tensor("out", [BC, 3], F32, kind="ExternalOutput").ap()

    with tile.TileContext(nc) as tc, ExitStack() as ctx:
        cpool = ctx.enter_context(tc.tile_pool(name="consts", bufs=1))
        hpool = ctx.enter_context(tc.tile_pool(name="hs", bufs=2))
        tpool = ctx.enter_context(tc.tile_pool(name="hst", bufs=2))
        apool = ctx.enter_context(tc.tile_pool(name="act", bufs=2))
        spool = ctx.enter_context(tc.tile_pool(name="small", bufs=2))
        stats = ctx.enter_context(tc.tile_pool(name="stats", bufs=1))
        pg_ps = ctx.enter_context(tc.tile_pool(name="pg", bufs=4, space="PSUM"))
        sm_ps = ctx.enter_context(tc.tile_pool(name="sm", bufs=2, space="PSUM"))
        asp_ps = ctx.enter_context(tc.tile_pool(name="asp", bufs=1, space="PSUM"))

        # ---- constants (loaded once) ----
        GW0 = cpool.tile([128, KT, H], F32, tag="gw0")
        nc.sync.dma_start(GW0[:], dr["gw"].rearrange("(k p) n -> p k n", p=128))
        GW = cpool.tile([128, KT, H], F32R, tag="gw")
        nc.vector.tensor_copy(GW[:], GW0[:])
        GBROW0 = cpool.tile([1, H], F32, tag="gbrow0")
        nc.sync.dma_start(GBROW0[:], dr["gbrow"][:])
        GBROW = cpool.tile([1, H], F32R, tag="gbrow")
        nc.vector.tensor_copy(GBROW[:], GBROW0[:])
        ONESR0 = cpool.tile([1, 128], F32, tag="onesrow0")
        nc.sync.dma_start(ONESR0[:], dr["onesrow"][:])
        ONESR = cpool.tile([1, 128], F32R, tag="onesrow")
        nc.vector.tensor_copy(ONESR[:], ONESR0[:])
        ROWSB = cpool.tile([1, BC * KC], F32, tag="rows")
        nc.sync.dma_start(ROWSB[:], dr["rows"][:])
        PIDX2 = cpool.tile([128, IT], F32, tag="pidx2")
        nc.sync.dma_start(PIDX2[:], dr["pidx2"][:])
        IOTA = cpool.tile([128, KC], F32, tag="iota")
        nc.sync.dma_start(IOTA[:], dr["iota"][:])
        EYE = cpool.tile([128, 128], F32, tag="eye")
        nc.sync.dma_start(EYE[:], dr["eye"][:])
        TSC = cpool.tile([128, IT, BC], F32, tag="tsc")
        nc.sync.dma_start(TSC[:], dr["tscT"].rearrange("(t p) s -> p t s", p=128))
        MT = cpool.tile([128, IT, BC], F32, tag="mt")
        nc.sync.dma_start(MT[:], dr["mT"].rearrange("(t p) s -> p t s", p=128))
        MN = cpool.tile([BC, L], F32, tag="mn")
        nc.sync.dma_start(MN[:], dr["mnat"][:])
        CLSW = cpool.tile([128, 5, 3], F32, tag="clsw")
        nc.sync.dma_start(CLSW[:], dr["clsw"].rearrange("(c p) n -> p c n", p=128))
        CLSB = cpool.tile([BC, 3], F32, tag="clsb")
        nc.sync.dma_start(CLSB[:], dr["clsb"][:])
        SREP = cpool.tile([BC, 3], F32, tag="srep")
        nc.sync.dma_start(SREP[:], dr["srep"][:])

        # 1/sum(m) per sample
        SM = stats.tile([BC, 1], F32, tag="sm")
        nc.vector.tensor_reduce(SM[:], MN[:], AX.X, ALU.add)
        RECIP = stats.tile([BC, 1], F32, tag="recip")
        nc.vector.reciprocal(RECIP[:], SM[:])

        # LN stats accumulators, one column per sample
        S1A = stats.tile([128, BC], F32, tag="s1a")
        S1B = stats.tile([128, BC], F32, tag="s1b")
        S2 = stats.tile([128, BC], F32, tag="s2")
        MU = stats.tile([128, BC], F32, tag="mu")
        RS = stats.tile([128, BC], F32, tag="rs")

        def body():
            ASPT = asp_ps.tile([128, 5 * BC], F32, tag="aspt")
            CPS = sm_ps.tile([1, BC], F32, tag="cps")
            for s in range(BC):
                # ---- load sample; gather+transpose to [d, j] compact ----
                HSN = hpool.tile([128, IT, D], F32, tag="hsn")
                nc.sync.dma_start(HSN[:], dr["hs"][s].rearrange("(t p) d -> p t d", p=128))
                RREP = spool.tile([128, KC], F32, tag="rrep")
                nc.gpsimd.partition_broadcast(RREP[:], ROWSB[0:1, s * KC:(s + 1) * KC])
                Gs = []
                for it in range(IT):
                    Git = spool.tile([128, KC], F32, tag="git")
                    nc.vector.tensor_scalar(Git[:], RREP[:], PIDX2[:, it:it + 1],
                                            None, ALU.is_equal)
                    Gs.append(Git)
                HST = tpool.tile([128, KT, KC], F32R, tag="hst")
                for kt in range(KT):
                    PT = pg_ps.tile([128, KC], F32, tag="pg")
                    for it in range(IT):
                        nc.tensor.matmul(
                            PT[:], HSN[:, it, kt * 128:(kt + 1) * 128], Gs[it][:],
                            start=(it == 0), stop=(it == IT - 1))
                    nc.vector.tensor_copy(HST[:, kt, :], PT[:])

                # ---- guidance matmul (float32r) + relu + stats ----
                GR2 = apool.tile([128, H], F32, tag="gr2")
                for ci, (nlo, nhi) in enumerate(NCH):
                    PG = pg_ps.tile([128, nhi - nlo], F32, tag="pg")
                    for kt in range(KT):
                        nc.tensor.matmul(
                            PG[:], HST[:, kt, :], GW[:, kt, nlo:nhi],
                            start=(kt == 0), stop=False)
                    nc.tensor.matmul(
                        PG[:], ONESR[:], GBROW[:, nlo:nhi], start=False, stop=True)
                    acc = (S1A if ci == 0 else S1B)[:, s:s + 1]
                    nc.scalar.activation(GR2[:, nlo:nhi], PG[:], ACTF.Relu,
                                         accum_out=acc)
                SQ = apool.tile([128, H], F32, tag="sq")
                nc.scalar.activation(SQ[:], GR2[:], ACTF.Square,
                                     accum_out=S2[:, s:s + 1])
                c1 = slice(s, s + 1)
                nc.vector.tensor_add(MU[:, c1], S1A[:, c1], S1B[:, c1])
                nc.vector.tensor_scalar_mul(MU[:, c1], MU[:, c1], 1.0 / H)
                V = spool.tile([128, 1], F32, tag="v")
                nc.vector.tensor_scalar_mul(V[:], S2[:, c1], 1.0 / H)
                MSQ = spool.tile([128, 1], F32, tag="msq")
                nc.vector.tensor_mul(MSQ[:], MU[:, c1], MU[:, c1])
                nc.vector.tensor_sub(V[:], V[:], MSQ[:])
                nc.vector.tensor_scalar_add(V[:], V[:], EPS)
                SD = spool.tile([128, 1], F32, tag="sd")
                nc.scalar.sqrt(SD[:], V[:])
                nc.vector.reciprocal(RS[:, c1], SD[:])

                # ---- gather weights w[j] = sum_i m[i][tsc[i]==j] ----
                WPS = sm_ps.tile([128, 1], F32, tag="cps")
                for it in range(IT):
                    SOH = spool.tile([128, KC], F32, tag="soh")
                    nc.vector.tensor_scalar(SOH[:], IOTA[:], TSC[:, it, s:s + 1],
                                            None, ALU.is_equal)
                    nc.tensor.matmul(
                        WPS[:], SOH[:], MT[:, it, s:s + 1],
                        start=(it == 0), stop=(it == IT - 1))
                # w2 = w * rstd (folds LN scale into the reduction weights)
                W2 = spool.tile([128, 1], F32, tag="w2")
                nc.vector.tensor_mul(W2[:], WPS[:], RS[:, c1])

                # ---- aspects^T column s + mean correction ----
                for hc, (hlo, hhi) in enumerate(HCH):
                    nc.tensor.matmul(
                        ASPT[:hhi - hlo, hc * BC + s:hc * BC + s + 1],
                        GR2[:, hlo:hhi], W2[:])
                nc.tensor.matmul(CPS[:, s:s + 1], MU[:, c1], W2[:])

            # -------- classifier --------
            ASB = stats.tile([128, 5 * BC], F32, tag="asb")
            for hc, (hlo, hhi) in enumerate(HCH):
                sz = hhi - hlo
                nc.scalar.copy(ASB[:sz, hc * BC:(hc + 1) * BC],
                               ASPT[:sz, hc * BC:(hc + 1) * BC])
            CROW = stats.tile([1, BC], F32, tag="crow")
            nc.vector.tensor_copy(CROW[:], CPS[:])
            CTP = sm_ps.tile([BC, 1], F32, tag="cps")
            nc.tensor.transpose(CTP[:], CROW[:], EYE[0:1, 0:1])
            CT = stats.tile([BC, 1], F32, tag="ct")
            nc.vector.tensor_copy(CT[:], CTP[:])

            LG = sm_ps.tile([BC, 3], F32, tag="cps")
            for hc, (hlo, hhi) in enumerate(HCH):
                sz = hhi - hlo
                nc.tensor.matmul(
                    LG[:], ASB[:sz, hc * BC:(hc + 1) * BC], CLSW[:sz, hc, :],
                    start=(hc == 0), stop=(hc == len(HCH) - 1))
            T1 = stats.tile([BC, 3], F32, tag="t1")
            nc.vector.tensor_scalar(T1[:], SREP[:], CT[:], None, ALU.mult)
            OSB = stats.tile([BC, 3], F32, tag="osb")
            nc.vector.tensor_sub(OSB[:], LG[:], T1[:])
            nc.vector.tensor_scalar(OSB[:], OSB[:], RECIP[:], None, ALU.mult)
            nc.vector.tensor_add(OSB[:], OSB[:], CLSB[:])
            nc.sync.dma_start(out_ap[:], OSB[:])

        if repeats == 1:
            body()
        else:
            with tc.For_i(0, repeats, 1):
                body()

    nc.compile()
    return nc


def host_inputs(inputs):
    """Slice/prepare per-core input maps from the full problem inputs.

    Host work is index bookkeeping only: compact row lists + one-hot
    comparison operands.  All tensor arithmetic happens on device.
    """
    hs12 = np.ascontiguousarray(np.asarray(inputs["hidden_states"])[12])  # [B,L,D]
    ts = np.asarray(inputs["token_starts"]).astype(np.int64)
    m = np.ascontiguousarray(np.asarray(inputs["aspect_in_text_mask"], dtype=np.float32))
    gw = np.ascontiguousarray(np.asarray(inputs["guid_W"], dtype=np.float32)[3])
    gb = np.asarray(inputs["guid_b"], dtype=np.float32)[3]
    ln_g = np.asarray(inputs["ln_g"], dtype=np.float32)
    ln_b = np.asarray(inputs["ln_b"], dtype=np.float32)
    cls_W = np.asarray(inputs["cls_W"], dtype=np.float32)
    cls_b = np.asarray(inputs["cls_b"], dtype=np.float32)

    clsw_eff = (ln_g[:, None] * cls_W).astype(np.float32)
    clsw_pad = np.zeros((640, 3), np.float32)
    clsw_pad[:H] = clsw_eff
    clsb_eff = (ln_b @ cls_W + cls_b).astype(np.float32)
    clsb_rep = np.tile(clsb_eff[None, :], (BC, 1)).astype(np.float32)
    srep = np.tile(clsw_eff.sum(0, dtype=np.float32)[None, :], (BC, 1)).astype(np.float32)
    iota = np.tile(np.arange(KC, dtype=np.float32)[None, :], (128, 1))
    eye = np.eye(128, dtype=np.float32)
    onesrow = np.ones((1, 128), np.float32)
    pidx2 = np.stack([np.arange(128, dtype=np.float32),
                      np.arange(128, dtype=np.float32) + 128], axis=1)
    pidx2 = np.ascontiguousarray(pidx2)

    # compact row lists (index bookkeeping)
    rows_all = np.zeros((B, KC), np.float32)
    tsc_all = np.zeros((B, L), np.float32)
    for b in range(B):
        used = np.unique(ts[b][m[b] > 0])
        assert len(used) <= KC, f"sample {b}: {len(used)} unique rows > {KC}"
        if len(used) < KC:
            # duplicate-pad with the first used row; padded one-hot columns
            # get w[j]=0 because tsc never points at them
            rows_all[b, :len(used)] = used.astype(np.float32)
            rows_all[b, len(used):] = -1.0
        else:
            rows_all[b] = used.astype(np.float32)
        lut = {int(v): j for j, v in enumerate(used)}
        for i in range(L):
            tsc_all[b, i] = lut.get(int(ts[b, i]), 0) if m[b, i] > 0 else 0
    in_maps = []
    for c in range(N_CORES):
        sl = slice(c * BC, (c + 1) * BC)
        in_maps.append(dict(
            hs=np.ascontiguousarray(hs12[sl]),
            gw=gw,
            gbrow=gb[None, :],
            onesrow=onesrow,
            rows=np.ascontiguousarray(rows_all[sl].reshape(1, BC * KC)),
            pidx2=pidx2,
            tscT=np.ascontiguousarray(tsc_all[sl].T),
            mT=np.ascontiguousarray(m[sl].T),
            mnat=np.ascontiguousarray(m[sl]),
            iota=iota,
            eye=eye,
            clsw=clsw_pad,
            clsb=clsb_rep,
            srep=srep,
        ))
    return in_maps


_PROGRAM = None


def kernel(**inputs):
    global _PROGRAM
    if _PROGRAM is None:
        _PROGRAM = build_program(repeats=1)
    nc = _PROGRAM
    in_maps = host_inputs(inputs)
    res = run_bass_kernel_spmd(nc, in_maps, list(range(N_CORES)), trace=False)
    out = np.concatenate([res.results[c]["out"] for c in range(N_CORES)], axis=0)
    return out.astype(np.float32)
